# revision 15
# baseline (speedup 1.0000x reference)
"""Expert-parallel MoE (top-2 of 8 experts, SwiGLU) on 8 Trainium2 NeuronCores.

Strategy: the router is 0.003% of the FLOPs, so it runs on the host along
with the dispatch bookkeeping; the device cores run the expert FFNs, which
is 99.99% of the work.  One expert per core (W1/W3/W2 sharded on the expert
axis).  The host computes top-2 routing, gathers each expert's tokens into a
dense, pre-transposed activation block, and each core runs a straight
gate/up/SwiGLU/down FFN over exactly max-count token slots (no fixed 640
capacity padding, no on-device top-k/compaction/indirect-gather machinery).
The host applies the combine weights and scatter-adds the per-expert
outputs back to token order (the unshard step for expert-parallel sharding).

Device kernel per core (identical program; per-core inputs):
  - xt   [128, 8k x CAP]  bf16: the expert's tokens, transposed (H on
          partitions), zero-padded to CAP.
  - w1/w3 [128, 16it x 8k x 128] bf16, w2 [128, 8h x 16k x 128] bf16,
          streamed in compute order so matmuls start ~2us in.
  - GU phase: for each of 16 I-tiles: gate/up matmuls (fp32 PSUM), fused
          Silu on ScalarE, one DVE multiply -> hT bf16 resident in SBUF.
  - D phase: for each of 8 H-tiles: down matmuls, evacuate fp16, DMA out.
All matmuls are N=512 (plus one short tail piece that reuses the same
weights immediately, so LDWEIGHTS stays hidden under the big matmul).
"""
import sys

sys.path.insert(0, "/opt/trn_rl_repo")

from contextlib import ExitStack

import ml_dtypes
import numpy as np

import concourse.bacc as bacc
import concourse.mybir as mybir
from concourse.bass_utils import run_bass_kernel_spmd
from concourse.tile import TileContext

F32 = mybir.dt.float32
BF16 = mybir.dt.bfloat16
FP16 = mybir.dt.float16
AF = mybir.ActivationFunctionType
OP = mybir.AluOpType

P = 128
B, S, H, I_DIM, E, TOP_K = 1, 2048, 1024, 2048, 8, 2
NKH = H // P    # 8 k-tiles over H
NIT = I_DIM // P  # 16 i-tiles (GU output m-tiles, D contraction k-tiles)
NHT = H // P    # 8 h-tiles (D output m-tiles)
N_CORES = 8


def _pieces(cap):
    """Column pieces of <=512 (one PSUM bank of fp32).  For cap in
    (512, 1024] use two near-halves: both pieces are then >=LDWEIGHTS
    hiding size (~230 cols) and per-matmul NX overhead is paid twice
    instead of once-big-once-tiny (which costs ~13ns extra per pair)."""
    if cap <= 512:
        return [(0, cap)]
    h = -(-cap // 8) * 4          # half, rounded up to a multiple of 4
    return [(0, h), (h, cap - h)]


def build_program(cap):
    nc = bacc.Bacc("TRN2", target_bir_lowering=False, debug=False,
                   num_devices=N_CORES)

    xt = nc.dram_tensor("xt", [P, NKH * cap], BF16, kind="ExternalInput")
    w1 = nc.dram_tensor("w1", [P, NIT * NKH * P], BF16, kind="ExternalInput")
    w3 = nc.dram_tensor("w3", [P, NIT * NKH * P], BF16, kind="ExternalInput")
    w2 = nc.dram_tensor("w2", [P, NHT * NIT * P], BF16, kind="ExternalInput")
    yt = nc.dram_tensor("yt", [H, cap], FP16, kind="ExternalOutput")

    pieces = _pieces(cap)

    with TileContext(nc) as tc, ExitStack() as ctx:
        const = ctx.enter_context(tc.tile_pool(name="const", bufs=1))
        wpool = ctx.enter_context(tc.tile_pool(name="wpool", bufs=1))
        xpool = ctx.enter_context(tc.tile_pool(name="xpool", bufs=1))
        hpool = ctx.enter_context(tc.tile_pool(name="hpool", bufs=1))
        spool = ctx.enter_context(tc.tile_pool(name="spool", bufs=3))
        epool = ctx.enter_context(tc.tile_pool(name="epool", bufs=3))

        # warmup stationary operand: content is irrelevant (outputs unused);
        # gpsimd finishes its preamble first of all engines (~6.5us), so a
        # gpsimd memset unblocks the first warmup transpose ~0.6us earlier
        # than a DVE memset would.
        id_bf = const.tile([P, P], BF16, tag="idb")
        nc.gpsimd.memset(id_bf[:], 1.0)

        # ---- DMA everything on one HWDGE queue, in consumption order:
        # first GU tile's weights, then xt k-slices (consumed in order by
        # the it=0 k-loop), then the remaining weights. ----
        xt_all = xpool.tile([P, NKH, cap], BF16, tag="xt")
        w1_all = wpool.tile([P, NIT, NKH, P], BF16, tag="w1")
        w3_all = wpool.tile([P, NIT, NKH, P], BF16, tag="w3")
        w2_all = wpool.tile([P, NHT, NIT, P], BF16, tag="w2")
        nc.sync.dma_start(out=w1_all[:, 0], in_=w1[:, 0:NKH * P])
        nc.sync.dma_start(out=xt_all[:, 0], in_=xt[:, 0:cap])
        nc.sync.dma_start(out=w3_all[:, 0], in_=w3[:, 0:NKH * P])
        for k in range(1, NKH):
            nc.sync.dma_start(
                out=xt_all[:, k], in_=xt[:, k * cap:(k + 1) * cap])
        # Everything stays on the sync HWDGE queue: descriptor generation is
        # ~0.65us per dma_start on the issuing engine, and the scalar engine
        # must stay free for the SwiGLU activations (queueing DMAs there
        # stalls the whole GU pipeline behind descriptor generation).
        for it in range(1, NIT):
            nc.sync.dma_start(
                out=w1_all[:, it], in_=w1[:, it * NKH * P:(it + 1) * NKH * P])
            nc.sync.dma_start(
                out=w3_all[:, it], in_=w3[:, it * NKH * P:(it + 1) * NKH * P])
        for h in range(NHT):
            nc.sync.dma_start(
                out=w2_all[:, h], in_=w2[:, h * NIT * P:(h + 1) * NIT * P])

        ht_all = hpool.tile([P, NIT, cap], BF16, tag="ht")

        assert len(pieces) <= 2 and cap <= 1024
        tail = pieces[1][1] if len(pieces) > 1 else 0

        with tc.tile_pool(name="psw", bufs=1, space="PSUM") as psw:
            # PE warmup: keep TensorE busy while the first DMAs land so HAM
            # un-throttles before the FFN matmuls start.
            # ~26 transposes bridge TensorE until the first weights + xt
            # k-slice land (~2.4us); more would queue ahead of the first
            # FFN matmuls on the PE FIFO and delay them.
            # The ~7us framework preamble delays the first DMA issue, so the
            # first FFN matmul can't start before ~10us; bridge TensorE from
            # ~7.3us to there so the HAM clock-gate window (3.4us of busy)
            # has fired and GU starts at full clock.
            warm_ps = psw.tile([P, P], BF16, tag="warm")
            for _ in range(30):
                nc.tensor.transpose(
                    out=warm_ps[:], in_=id_bf[:], identity=id_bf[:])

        # PSUM budget is 8 banks of 2 KiB, allocated bank-granular.  Every
        # accumulator gets a full private bank (tiles are all [P, 512] fp32
        # even for the short tail piece) because a group's start=True clears
        # has_written bits for its WHOLE bank - co-resident accumulators
        # would corrupt each other when their k-loops interleave.  4 tags x
        # bufs=2 = 8 banks; the D phase reuses the GU tags.
        with tc.tile_pool(name="ps", bufs=2, space="PSUM") as ps:
            TAGS = (("g", "u"), ("ta", "tb"))  # (big tags, tail tags)

            def piece_tiles(which, name):
                tiles = [ps.tile([P, 512], F32, tag=TAGS[0][which],
                                 name=f"{TAGS[0][which]}_{name}")]
                if tail:
                    tiles.append(ps.tile([P, 512], F32, tag=TAGS[1][which],
                                         name=f"{TAGS[1][which]}_{name}"))
                return tiles

            def piece_ap(tiles, pi):
                return tiles[pi][:, :pieces[pi][1]]

            # ---- GU phase: gate/up + SwiGLU -> hT ----
            # gate and up alternate per k-tile so every LDWEIGHTS hides
            # under the preceding N=512 matmul's streaming time.
            for it in range(NIT):
                gps = piece_tiles(0, f"g{it}")
                ups = piece_tiles(1, f"u{it}")
                for k in range(NKH):
                    for tiles, w_all in ((gps, w1_all), (ups, w3_all)):
                        for pi, (c0, n) in enumerate(pieces):
                            nc.tensor.matmul(
                                out=piece_ap(tiles, pi),
                                lhsT=w_all[:, it, k, :],
                                rhs=xt_all[:, k, c0:c0 + n],
                                start=(k == 0), stop=(k == NKH - 1))
                sl = spool.tile([P, cap], BF16, tag="sl", name=f"sl_{it}")
                for pi, (c0, n) in enumerate(pieces):
                    nc.scalar.activation(
                        out=sl[:, c0:c0 + n], in_=piece_ap(gps, pi),
                        func=AF.Silu)
                for pi, (c0, n) in enumerate(pieces):
                    nc.vector.tensor_tensor(
                        out=ht_all[:, it, c0:c0 + n], in0=sl[:, c0:c0 + n],
                        in1=piece_ap(ups, pi), op=OP.mult)

            # ---- D phase: down -> yT fp16 -> DRAM ----
            for h in range(NHT):
                yps = piece_tiles(h % 2, f"y{h}")
                for k in range(NIT):
                    for pi, (c0, n) in enumerate(pieces):
                        nc.tensor.matmul(
                            out=piece_ap(yps, pi), lhsT=w2_all[:, h, k, :],
                            rhs=ht_all[:, k, c0:c0 + n],
                            start=(k == 0), stop=(k == NIT - 1))
                ysb = epool.tile([P, cap], FP16, tag="ysb", name=f"ysb_{h}")
                for pi, (c0, n) in enumerate(pieces):
                    # split evacuation across ScalarE and VectorE
                    if pi % 2 == 0:
                        nc.scalar.activation(
                            out=ysb[:, c0:c0 + n], in_=piece_ap(yps, pi),
                            func=AF.Copy)
                    else:
                        nc.vector.tensor_copy(
                            out=ysb[:, c0:c0 + n], in_=piece_ap(yps, pi))
                nc.sync.dma_start(out=yt[h * P:(h + 1) * P, :], in_=ysb[:])

    nc.compile()
    return nc


_PROGRAMS = {}


def _get_program(cap):
    if cap not in _PROGRAMS:
        _PROGRAMS[cap] = build_program(cap)
    return _PROGRAMS[cap]


def _route(x2d, Wr, br):
    """Host router: top-2 selection + softmax combine weights."""
    logits = x2d.astype(np.float32) @ Wr.astype(np.float32) + br
    order = np.argsort(-logits, axis=1, kind="stable")
    top2 = order[:, :TOP_K]                       # [S, 2] expert ids
    l12 = np.take_along_axis(logits, top2, axis=1)
    m = l12.max(axis=1, keepdims=True)
    ex = np.exp(l12 - m)
    rw = ex / ex.sum(axis=1, keepdims=True)       # [S, 2] combine weights
    return top2, rw


def _prepare(x, Wr, br, W1, W3, W2):
    x2d = np.asarray(x, dtype=np.float32).reshape(S, H)
    top2, rw = _route(x2d, np.asarray(Wr, np.float32),
                      np.asarray(br, np.float32))
    toks, cs = [], []
    for e in range(E):
        sel = top2 == e                           # [S, 2]
        mask = sel.any(axis=1)
        tok_e = np.nonzero(mask)[0]
        c_e = rw[mask][sel[mask]]                 # weight of expert e per tok
        toks.append(tok_e)
        cs.append(c_e.astype(np.float32))
    n_max = max(len(t) for t in toks)
    cap = -(-n_max // 4) * 4                      # pad to multiple of 4
    xbf = x2d.astype(ml_dtypes.bfloat16)
    W1 = np.asarray(W1, np.float32)
    W3 = np.asarray(W3, np.float32)
    W2 = np.asarray(W2, np.float32)
    in_maps = []
    for e in range(E):
        xg = np.zeros((cap, H), ml_dtypes.bfloat16)
        xg[:len(toks[e])] = xbf[toks[e]]
        # [cap, H] -> [P, NKH, cap] -> [P, NKH*cap]
        xt = np.ascontiguousarray(
            xg.T.reshape(NKH, P, cap).transpose(1, 0, 2).reshape(P, -1))
        w1p = np.ascontiguousarray(
            W1[e].astype(ml_dtypes.bfloat16)
            .reshape(NKH, P, NIT, P).transpose(1, 2, 0, 3).reshape(P, -1))
        w3p = np.ascontiguousarray(
            W3[e].astype(ml_dtypes.bfloat16)
            .reshape(NKH, P, NIT, P).transpose(1, 2, 0, 3).reshape(P, -1))
        w2p = np.ascontiguousarray(
            W2[e].astype(ml_dtypes.bfloat16)
            .reshape(NIT, P, NHT, P).transpose(1, 2, 0, 3).reshape(P, -1))
        in_maps.append({"xt": xt, "w1": w1p, "w3": w3p, "w2": w2p})
    return in_maps, toks, cs, cap


def _combine(results, toks, cs):
    out = np.zeros((S, H), np.float32)
    for e in range(E):
        n_e = len(toks[e])
        if n_e == 0:
            continue
        yt = np.asarray(results[e]["yt"]).astype(np.float32)  # [H, cap]
        out[toks[e]] += cs[e][:, None] * yt[:, :n_e].T
    return out.reshape(B, S, H)


def run_on_device(inputs, trace=False, trace_cores=None):
    """Run the SPMD program; returns (full_output, BassKernelResults)."""
    in_maps, toks, cs, cap = _prepare(**inputs)
    nc = _get_program(cap)
    kwargs = {}
    if trace:
        try:
            import types

            if "antenv.axon_hooks" not in sys.modules:
                from trn_agent_boot.trn_boot import _ntff_profile_via_ctypes

                hook = _ntff_profile_via_ctypes("/opt/axon/libaxon_pjrt.so")
                mod = types.ModuleType("antenv.axon_hooks")
                mod._hook = hook
                mod.get_axon_ntff_profile_hook = lambda: mod._hook

                def _set(h):
                    mod._hook = h

                mod.set_axon_ntff_profile_hook = _set
                sys.modules["antenv.axon_hooks"] = mod
                import antenv

                antenv.axon_hooks = mod
        except Exception as exc:  # profiling unavailable -> run untraced
            print(f"trace hook install failed: {exc}", file=sys.stderr)
        kwargs = dict(trace=True,
                      trace_cores=trace_cores or list(range(N_CORES)))
    res = run_bass_kernel_spmd(nc, in_maps, list(range(N_CORES)), **kwargs)
    return _combine(res.results, toks, cs), res


def kernel(x, Wr, br, W1, W3, W2):
    out, _ = run_on_device(dict(x=x, Wr=Wr, br=br, W1=W1, W3=W3, W2=W2))
    return out
